# revision 10
# baseline (speedup 1.0000x reference)
"""Distributed causal-attention block (dense_transformer) on 8 TRN2 NeuronCores.

Sharding: data-parallel over batch (b=2) x tensor-parallel over head pairs
(8 heads -> 4 groups of 2). Core i handles batch i//4, heads (2*(i%4), 2*(i%4)+1).

Per-core pipeline (software-pipelined across the 8 q tiles):
  - token-chunked QKV projections (transposed layouts; V natural+ones column)
  - block-causal flash-style attention (S^T = K @ Q^T, denominator via the
    augmented ones-column in V); exp on the scalar engine, diagonal causal
    masks on the (otherwise idle) gpsimd engine
  - per-tile softmax normalization on the sender: l broadcast across
    partitions via a K=1 fp32 matmul into a recycled PSUM tile, fast
    reciprocal, one tensor_mul per head
  - AllToAll (4x less wire traffic than ReduceScatter of partial O-proj
    sums) redistributes normalized head outputs so each core owns a token
    quarter with all 512 head dims, then a local O projection + bias.

B, S, D, H = 2, 4096, 512, 8 (hd=64). Hardcoded per problem spec.
"""

import numpy as np
import ml_dtypes

import concourse.bacc as bacc
import concourse.mybir as mybir
from concourse import tile
from concourse.bass_utils import run_bass_kernel_spmd

B, S, D = 2, 4096, 512
H = 8
HD = D // H          # 64
NCORES = 8
R = 128              # qkv rows per core (2 heads x 64)
NT = 8               # q tiles of 512
QW = 512             # q tile width

BF16 = mybir.dt.bfloat16
F32 = mybir.dt.float32
AF = mybir.ActivationFunctionType
BF16_NP = ml_dtypes.bfloat16

_CACHE = {}


def _build_nc():
    nc = bacc.Bacc(num_devices=NCORES)

    xT = nc.declare_dram_parameter("xT", [D, S], BF16, isOutput=False)
    wqT = nc.declare_dram_parameter("wqT", [D, R], BF16, isOutput=False)
    wkT = nc.declare_dram_parameter("wkT", [D, R], BF16, isOutput=False)
    wvT = nc.declare_dram_parameter("wvT", [D, 130], BF16, isOutput=False)
    bq = nc.declare_dram_parameter("bq", [R, 1], F32, isOutput=False)
    bk = nc.declare_dram_parameter("bk", [R, 1], F32, isOutput=False)
    bvb = nc.declare_dram_parameter("bvb", [128, 130], F32, isOutput=False)
    woT = nc.declare_dram_parameter("woT", [D, D], BF16, isOutput=False)
    bob = nc.declare_dram_parameter("bob", [128, D], F32, isOutput=False)
    maskt = nc.declare_dram_parameter("maskt", [128, 128], BF16, isOutput=False)
    ones = nc.declare_dram_parameter("ones", [1, HD], F32, isOutput=False)
    out_ext = nc.declare_dram_parameter("out", [NT * 128, D], F32, isOutput=True)

    # AllToAll payload: chunk j = this core's 128 head-dims for tile tokens
    # 64j..64(j+1); rec[i] = 128 head-dims from global rank i (ranks 0-3 are
    # batch 0, ranks 4-7 batch 1) for this core's 64-token slice.
    parts = [nc.dram_tensor(f"part{t}", [8, 128, 64], BF16) for t in range(NT)]
    recs = [nc.dram_tensor(f"rec{t}", [8, 128, 64], BF16) for t in range(NT)]

    with tile.TileContext(nc) as tc:
        with (
            tc.tile_pool(name="const", bufs=1) as cpool,
            tc.tile_pool(name="xres", bufs=1) as xpool,
            tc.tile_pool(name="pt", bufs=12) as ppool,
            tc.tile_pool(name="small", bufs=3) as spool,
            tc.tile_pool(name="stage", bufs=3) as stpool,
            tc.tile_pool(name="ps_s", bufs=2, space="PSUM") as ps_s,
            tc.tile_pool(name="ps_o", bufs=1, space="PSUM") as ps_o,
            tc.tile_pool(name="ps_p", bufs=1, space="PSUM") as ps_p,
        ):
            # ---------- weights / constants into SBUF (attention-critical first)
            wq_sb = cpool.tile([128, D], BF16)
            nc.sync.dma_start(wq_sb[:].rearrange("p (c m) -> p c m", c=4),
                              wqT[:, :].rearrange("(c p) m -> p c m", p=128))
            wk_sb = cpool.tile([128, D], BF16)
            nc.sync.dma_start(wk_sb[:].rearrange("p (c m) -> p c m", c=4),
                              wkT[:, :].rearrange("(c p) m -> p c m", p=128))
            wv_sb = cpool.tile([128, 4 * 130], BF16)
            nc.sync.dma_start(wv_sb[:].rearrange("p (c m) -> p c m", c=4),
                              wvT[:, :].rearrange("(c p) m -> p c m", p=128))
            bq_sb = cpool.tile([R, 1], F32)
            nc.sync.dma_start(bq_sb[:], bq[:, :])
            bk_sb = cpool.tile([R, 1], F32)
            nc.sync.dma_start(bk_sb[:], bk[:, :])
            bvb_sb = cpool.tile([128, 130], F32)
            nc.sync.dma_start(bvb_sb[:], bvb[:, :])
            mask_sb = cpool.tile([128, 128], BF16)
            nc.sync.dma_start(mask_sb[:], maskt[:, :])
            ones_sb = cpool.tile([1, HD], F32)
            nc.sync.dma_start(ones_sb[:], ones[:, :])

            xt = [xpool.tile([128, S], BF16, tag=f"xt{c}", name=f"xt{c}")
                  for c in range(4)]
            qT = xpool.tile([128, S], BF16, tag="qT")
            kT = xpool.tile([128, S], BF16, tag="kT")
            vaug = xpool.tile([128, 32 * 130], BF16, tag="vaug")

            def load_x(nt):
                for c in range(4):
                    nc.sync.dma_start(
                        xt[c][:, QW * nt:QW * (nt + 1)],
                        xT[128 * c:128 * (c + 1), QW * nt:QW * (nt + 1)],
                    )

            load_x(0)
            load_x(1)
            # O-proj weights needed only from the first epilogue onward
            wo_sb = cpool.tile([128, 4 * D], BF16)
            nc.sync.dma_start(wo_sb[:].rearrange("p (g m) -> p g m", g=4),
                              woT[:, :].rearrange("(g p) m -> p g m", p=128))
            bob_sb = cpool.tile([128, D], F32)
            nc.sync.dma_start(bob_sb[:], bob[:, :])
            for nt in range(2, NT):
                load_x(nt)

            def proj(nt):
                # Q^T / K^T rows for token block nt
                for w_sb, b_sb, dst in ((wq_sb, bq_sb, qT), (wk_sb, bk_sb, kT)):
                    ps = ps_p.tile([128, QW], F32, tag="po")
                    for c in range(4):
                        nc.tensor.matmul(
                            ps[:],
                            w_sb[:, 128 * c:128 * (c + 1)],
                            xt[c][:, QW * nt:QW * (nt + 1)],
                            start=(c == 0), stop=(c == 3),
                        )
                    nc.vector.tensor_scalar_add(
                        dst[:, QW * nt:QW * (nt + 1)], ps[:], b_sb[:])
                # V natural (augmented with ones col per head)
                for tb in range(4 * nt, 4 * (nt + 1)):
                    ps = ps_p.tile([128, QW], F32, tag="po")
                    for c in range(4):
                        nc.tensor.matmul(
                            ps[:, 0:130],
                            xt[c][:, 128 * tb:128 * (tb + 1)],
                            wv_sb[:, 130 * c:130 * (c + 1)],
                            start=(c == 0), stop=(c == 3),
                        )
                    nc.vector.tensor_add(
                        vaug[:, 130 * tb:130 * (tb + 1)], ps[:, 0:130], bvb_sb[:]
                    )

            def att_jloop(t):
                nj = 4 * t + 4          # causal: k blocks 0 .. 4t+3
                o0 = ps_o.tile([128, QW], F32, tag="o0")
                o1 = ps_o.tile([128, QW], F32, tag="o1")
                for j in range(nj):
                    # causal: q columns < q0 are fully masked for this k block
                    q0 = max(0, 128 * (j - 4 * t))
                    s = ps_s.tile([128, 2 * QW], F32, tag="s")
                    for h in (0, 1):
                        nc.tensor.matmul(
                            s[:, QW * h + q0:QW * (h + 1)],
                            kT[64 * h:64 * (h + 1), 128 * j:128 * (j + 1)],
                            qT[64 * h:64 * (h + 1), QW * t + q0:QW * (t + 1)],
                            start=True, stop=True,
                        )
                    p = ppool.tile([128, 2 * QW], BF16, tag="p")
                    if q0 == 0:
                        nc.scalar.activation(p[:], s[:], AF.Exp, bias=0.0, scale=0.125)
                    else:
                        sv = s[:].rearrange("k (h q) -> k h q", h=2)[:, :, q0:QW]
                        pv = p[:].rearrange("k (h q) -> k h q", h=2)[:, :, q0:QW]
                        nc.scalar.activation(pv, sv, AF.Exp, bias=0.0, scale=0.125)
                    if j >= 4 * t:  # diagonal 128-col boundary: 0/1 mask
                        for h in (0, 1):
                            nc.gpsimd.tensor_mul(
                                p[:, QW * h + q0:QW * h + q0 + 128],
                                p[:, QW * h + q0:QW * h + q0 + 128],
                                mask_sb[:, :],
                            )
                    for h, oo in ((0, o0), (1, o1)):
                        nc.tensor.matmul(
                            oo[0:65, q0:QW],
                            vaug[:, 130 * j + 65 * h:130 * j + 65 * (h + 1)],
                            p[:, QW * h + q0:QW * (h + 1)],
                            start=(j == 0), stop=(j == nj - 1),
                        )
                return o0, o1

            def norm_send(t, o0, o1):
                # l rows (row 64 of o0/o1) -> SBUF, broadcast across
                # partitions via K=1 fp32 matmuls into a recycled s-tile,
                # reciprocal, then scale O^T and ship to the AllToAll.
                lrow = spool.tile([1, 2 * QW], F32, tag="lrow")
                nc.vector.tensor_copy(lrow[0:1, 0:QW], o0[64:65, :])
                nc.vector.tensor_copy(lrow[0:1, QW:2 * QW], o1[64:65, :])
                lb = ps_s.tile([128, 2 * QW], F32, tag="s")
                for h in range(2):
                    nc.tensor.matmul(
                        lb[0:64, QW * h:QW * (h + 1)],
                        ones_sb[0:1, :],
                        lrow[0:1, QW * h:QW * (h + 1)],
                        start=True, stop=True,
                    )
                linv = spool.tile([64, 2 * QW], F32, tag="linv")
                nc.vector.reciprocal_approx_fast(linv[:], lb[0:64, :])
                ocn0 = spool.tile([64, QW], BF16, tag="ocn0")
                ocn1 = spool.tile([64, QW], BF16, tag="ocn1")
                nc.vector.tensor_mul(ocn0[:], o0[0:64, :], linv[:, 0:QW])
                nc.vector.tensor_mul(ocn1[:], o1[0:64, :], linv[:, QW:2 * QW])
                nc.sync.dma_start(
                    parts[t][:, 0:64, :].rearrange("r p c -> p r c"),
                    ocn0[:].rearrange("p (r c) -> p r c", r=8),
                )
                nc.sync.dma_start(
                    parts[t][:, 64:128, :].rearrange("r p c -> p r c"),
                    ocn1[:].rearrange("p (r c) -> p r c", r=8),
                )
                nc.gpsimd.collective_compute(
                    "AllToAll",
                    mybir.AluOpType.bypass,
                    replica_groups=[[0, 1, 2, 3, 4, 5, 6, 7]],
                    ins=[parts[t][:, :, :]],
                    outs=[recs[t][:, :, :]],
                )

            def recv_oproj(t):
                # ysb cols 128g+[0:64] = batch-0 sender g, +[64:128] = batch-1
                # sender g: each 128-col block is one lhsT (same Wo rows).
                ysb = stpool.tile([128, D], BF16, tag="ysb")
                for b in range(2):
                    nc.sync.dma_start(
                        ysb[:].rearrange("p (g b c) -> b p g c", g=4, b=2)[b],
                        recs[t][4 * b:4 * (b + 1), :, :].rearrange(
                            "g p c -> p g c"),
                    )
                po = ps_p.tile([128, D], F32, tag="pr")
                for g in range(4):
                    nc.tensor.matmul(
                        po[:],
                        ysb[:, 128 * g:128 * (g + 1)],
                        wo_sb[:, D * g:D * (g + 1)],
                        start=(g == 0), stop=(g == 3),
                    )
                ost = stpool.tile([128, D], F32, tag="ost")
                nc.vector.tensor_add(ost[:], po[:], bob_sb[:])
                nc.sync.dma_start(out_ext[128 * t:128 * (t + 1), :], ost[:])

            proj(0)
            for t in range(NT):
                o0, o1 = att_jloop(t)
                norm_send(t, o0, o1)
                if t + 1 < NT:
                    proj(t + 1)
                if t >= 1:
                    recv_oproj(t - 1)
            recv_oproj(NT - 1)

    nc.finalize()
    return nc


def _make_in_maps(x, Wqkv, bqkv, Wo, bo):
    # causal 0/1 multiplicative mask for the diagonal 128x128 sub-block:
    # keep (p, o) where o >= p (k = block_base + p, q = block_base + o)
    p_idx = np.arange(128)[:, None]
    o_idx = np.arange(128)[None, :]
    maskt = (o_idx >= p_idx).astype(np.float32).astype(BF16_NP)

    in_maps = []
    for core in range(NCORES):
        b = core // 4
        g = core % 4
        rows = slice(128 * g, 128 * (g + 1))
        wq = Wqkv[0:D][rows]            # [128, 512]
        wk = Wqkv[D:2 * D][rows]
        wv = Wqkv[2 * D:3 * D][rows]
        wvT = np.zeros((D, 130), dtype=np.float32)
        wvT[:, 0:64] = wv[0:64].T
        wvT[:, 65:129] = wv[64:128].T
        bvb = np.zeros((128, 130), dtype=np.float32)
        bvb[:, 0:64] = bqkv[2 * D:3 * D][rows][0:64][None, :]
        bvb[:, 64] = 1.0
        bvb[:, 65:129] = bqkv[2 * D:3 * D][rows][64:128][None, :]
        bvb[:, 129] = 1.0
        in_maps.append({
            "xT": np.ascontiguousarray(x[b].T).astype(BF16_NP),
            "wqT": np.ascontiguousarray(wq.T).astype(BF16_NP),
            "wkT": np.ascontiguousarray(wk.T).astype(BF16_NP),
            "wvT": wvT.astype(BF16_NP),
            "bq": np.ascontiguousarray(bqkv[0:D][rows][:, None]).astype(np.float32),
            "bk": np.ascontiguousarray(bqkv[D:2 * D][rows][:, None]).astype(np.float32),
            "bvb": bvb,
            "woT": np.ascontiguousarray(Wo.T).astype(BF16_NP),
            "bob": np.tile(bo.astype(np.float32)[None, :], (128, 1)),
            "maskt": maskt,
            "ones": np.ones((1, HD), dtype=np.float32),
        })
    return in_maps


def run(x, Wqkv, bqkv, Wo, bo, trace=False):
    if "nc" not in _CACHE:
        _CACHE["nc"] = _build_nc()
    nc = _CACHE["nc"]
    in_maps = _make_in_maps(x, Wqkv, bqkv, Wo, bo)
    res = run_bass_kernel_spmd(nc, in_maps, core_ids=list(range(NCORES)), trace=trace)
    out = np.empty((B, S, D), dtype=np.float32)
    for core in range(NCORES):
        o = res.results[core]["out"]
        # 8-way AllToAll: core j owns tile-t tokens 512t+64j..+64 for BOTH
        # batches (rows 128t..+64 = batch 0, rows 128t+64..+128 = batch 1)
        for t in range(NT):
            tok = QW * t + 64 * core
            out[0, tok:tok + 64, :] = o[128 * t:128 * t + 64]
            out[1, tok:tok + 64, :] = o[128 * t + 64:128 * (t + 1)]
    return out, res


def kernel(x, Wqkv, bqkv, Wo, bo):
    out, _ = run(np.asarray(x, dtype=np.float32), np.asarray(Wqkv, dtype=np.float32),
                 np.asarray(bqkv, dtype=np.float32), np.asarray(Wo, dtype=np.float32),
                 np.asarray(bo, dtype=np.float32))
    return out


# revision 13
# speedup vs baseline: 1.0210x; 1.0210x over previous
"""Distributed causal-attention block (dense_transformer) on 8 TRN2 NeuronCores.

Sharding: data-parallel over batch (b=2) x tensor-parallel over head pairs
(8 heads -> 4 groups of 2). Core i handles batch i//4, heads (2*(i%4), 2*(i%4)+1).

Per-core pipeline (software-pipelined across the 8 q tiles):
  - token-chunked QKV projections (transposed layouts; V natural+ones column)
  - block-causal flash-style attention (S^T = K @ Q^T, denominator via the
    augmented ones-column in V); exp on the scalar engine, diagonal causal
    masks on the (otherwise idle) gpsimd engine
  - per-tile softmax normalization on the sender: l broadcast across
    partitions via a K=1 fp32 matmul into a recycled PSUM tile, fast
    reciprocal, one tensor_mul per head
  - AllToAll (4x less wire traffic than ReduceScatter of partial O-proj
    sums) redistributes normalized head outputs so each core owns a token
    quarter with all 512 head dims, then a local O projection + bias.

B, S, D, H = 2, 4096, 512, 8 (hd=64). Hardcoded per problem spec.
"""

import numpy as np
import ml_dtypes

import concourse.bacc as bacc
import concourse.mybir as mybir
from concourse import tile
from concourse.bass_utils import run_bass_kernel_spmd

B, S, D = 2, 4096, 512
H = 8
HD = D // H          # 64
NCORES = 8
R = 128              # qkv rows per core (2 heads x 64)
NT = 8               # q tiles of 512
QW = 512             # q tile width

BF16 = mybir.dt.bfloat16
F32 = mybir.dt.float32
AF = mybir.ActivationFunctionType
BF16_NP = ml_dtypes.bfloat16

_CACHE = {}


def _build_nc():
    nc = bacc.Bacc(num_devices=NCORES)

    xT = nc.declare_dram_parameter("xT", [D, S], BF16, isOutput=False)
    wqT = nc.declare_dram_parameter("wqT", [D, R], BF16, isOutput=False)
    wkT = nc.declare_dram_parameter("wkT", [D, R], BF16, isOutput=False)
    wvT = nc.declare_dram_parameter("wvT", [D, 130], BF16, isOutput=False)
    bq = nc.declare_dram_parameter("bq", [R, 1], F32, isOutput=False)
    bk = nc.declare_dram_parameter("bk", [R, 1], F32, isOutput=False)
    bvb = nc.declare_dram_parameter("bvb", [128, 130], F32, isOutput=False)
    woT = nc.declare_dram_parameter("woT", [D, D], BF16, isOutput=False)
    bob = nc.declare_dram_parameter("bob", [128, D], F32, isOutput=False)
    maskt = nc.declare_dram_parameter("maskt", [128, 128], BF16, isOutput=False)
    ones = nc.declare_dram_parameter("ones", [1, HD], BF16, isOutput=False)
    out_ext = nc.declare_dram_parameter("out", [NT * 128, D], F32, isOutput=True)

    # AllToAll payload, grouped to amortize the ~13us fixed collective cost:
    # groups of q tiles [0-4], [5-6], [7]. Per tile, chunk j = this core's 128
    # head-dims for tile tokens 64j..64(j+1), flattened [p, c] -> p*64+c;
    # rec row i = the same from global rank i (ranks 0-3 batch 0, 4-7 batch 1).
    GROUPS = [(0, 1, 2, 3, 4), (5, 6), (7,)]
    TILE_FLAT = 128 * 64
    group_of = {t: gi for gi, g in enumerate(GROUPS) for t in g}
    parts = [nc.dram_tensor(f"part{gi}", [8, len(g) * TILE_FLAT], BF16)
             for gi, g in enumerate(GROUPS)]
    recs = [nc.dram_tensor(f"rec{gi}", [8, len(g) * TILE_FLAT], BF16)
            for gi, g in enumerate(GROUPS)]

    with tile.TileContext(nc) as tc:
        with (
            tc.tile_pool(name="const", bufs=1) as cpool,
            tc.tile_pool(name="xres", bufs=1) as xpool,
            tc.tile_pool(name="pt", bufs=12) as ppool,
            tc.tile_pool(name="small", bufs=3) as spool,
            tc.tile_pool(name="stage", bufs=3) as stpool,
            tc.tile_pool(name="ps_s", bufs=2, space="PSUM") as ps_s,
            tc.tile_pool(name="ps_o", bufs=1, space="PSUM") as ps_o,
            tc.tile_pool(name="ps_p", bufs=1, space="PSUM") as ps_p,
        ):
            # ---------- weights / constants into SBUF (attention-critical first)
            wq_sb = cpool.tile([128, D], BF16)
            nc.sync.dma_start(wq_sb[:].rearrange("p (c m) -> p c m", c=4),
                              wqT[:, :].rearrange("(c p) m -> p c m", p=128))
            wk_sb = cpool.tile([128, D], BF16)
            nc.sync.dma_start(wk_sb[:].rearrange("p (c m) -> p c m", c=4),
                              wkT[:, :].rearrange("(c p) m -> p c m", p=128))
            wv_sb = cpool.tile([128, 4 * 130], BF16)
            nc.sync.dma_start(wv_sb[:].rearrange("p (c m) -> p c m", c=4),
                              wvT[:, :].rearrange("(c p) m -> p c m", p=128))
            bq_sb = cpool.tile([R, 1], F32)
            nc.sync.dma_start(bq_sb[:], bq[:, :])
            bk_sb = cpool.tile([R, 1], F32)
            nc.sync.dma_start(bk_sb[:], bk[:, :])
            bvb_sb = cpool.tile([128, 130], F32)
            nc.sync.dma_start(bvb_sb[:], bvb[:, :])
            mask_sb = cpool.tile([128, 128], BF16)
            nc.sync.dma_start(mask_sb[:], maskt[:, :])
            ones_sb = cpool.tile([1, HD], BF16)
            nc.sync.dma_start(ones_sb[:], ones[:, :])

            xt = [xpool.tile([128, S], BF16, tag=f"xt{c}", name=f"xt{c}")
                  for c in range(4)]
            qT = xpool.tile([128, S], BF16, tag="qT")
            kT = xpool.tile([128, S], BF16, tag="kT")
            vaug = xpool.tile([128, 32 * 130], BF16, tag="vaug")

            def load_x(nt):
                for c in range(4):
                    nc.sync.dma_start(
                        xt[c][:, QW * nt:QW * (nt + 1)],
                        xT[128 * c:128 * (c + 1), QW * nt:QW * (nt + 1)],
                    )

            load_x(0)
            load_x(1)
            # O-proj weights needed only from the first epilogue onward
            wo_sb = cpool.tile([128, 4 * D], BF16)
            nc.sync.dma_start(wo_sb[:].rearrange("p (g m) -> p g m", g=4),
                              woT[:, :].rearrange("(g p) m -> p g m", p=128))
            bob_sb = cpool.tile([128, D], F32)
            nc.sync.dma_start(bob_sb[:], bob[:, :])
            for nt in range(2, NT):
                load_x(nt)

            def proj(nt):
                # Q^T / K^T rows for token block nt
                for w_sb, b_sb, dst in ((wq_sb, bq_sb, qT), (wk_sb, bk_sb, kT)):
                    ps = ps_p.tile([128, QW], F32, tag="po")
                    for c in range(4):
                        nc.tensor.matmul(
                            ps[:],
                            w_sb[:, 128 * c:128 * (c + 1)],
                            xt[c][:, QW * nt:QW * (nt + 1)],
                            start=(c == 0), stop=(c == 3),
                        )
                    nc.vector.tensor_scalar_add(
                        dst[:, QW * nt:QW * (nt + 1)], ps[:], b_sb[:])
                # V natural (augmented with ones col per head)
                for tb in range(4 * nt, 4 * (nt + 1)):
                    ps = ps_p.tile([128, QW], F32, tag="po")
                    for c in range(4):
                        nc.tensor.matmul(
                            ps[:, 0:130],
                            xt[c][:, 128 * tb:128 * (tb + 1)],
                            wv_sb[:, 130 * c:130 * (c + 1)],
                            start=(c == 0), stop=(c == 3),
                        )
                    nc.vector.tensor_add(
                        vaug[:, 130 * tb:130 * (tb + 1)], ps[:, 0:130], bvb_sb[:]
                    )

            def att_jloop(t):
                nj = 4 * t + 4          # causal: k blocks 0 .. 4t+3
                o0 = ps_o.tile([128, QW], F32, tag="o0")
                o1 = ps_o.tile([128, QW], F32, tag="o1")
                for j in range(nj):
                    # causal: q columns < q0 are fully masked for this k block
                    q0 = max(0, 128 * (j - 4 * t))
                    s = ps_s.tile([128, 2 * QW], F32, tag="s")
                    for h in (0, 1):
                        nc.tensor.matmul(
                            s[:, QW * h + q0:QW * (h + 1)],
                            kT[64 * h:64 * (h + 1), 128 * j:128 * (j + 1)],
                            qT[64 * h:64 * (h + 1), QW * t + q0:QW * (t + 1)],
                            start=True, stop=True,
                        )
                    p = ppool.tile([128, 2 * QW], BF16, tag="p")
                    if q0 == 0:
                        nc.scalar.activation(p[:], s[:], AF.Exp, bias=0.0, scale=0.125)
                    else:
                        sv = s[:].rearrange("k (h q) -> k h q", h=2)[:, :, q0:QW]
                        pv = p[:].rearrange("k (h q) -> k h q", h=2)[:, :, q0:QW]
                        nc.scalar.activation(pv, sv, AF.Exp, bias=0.0, scale=0.125)
                    if j >= 4 * t:  # diagonal 128-col boundary: 0/1 mask
                        for h in (0, 1):
                            nc.gpsimd.tensor_mul(
                                p[:, QW * h + q0:QW * h + q0 + 128],
                                p[:, QW * h + q0:QW * h + q0 + 128],
                                mask_sb[:, :],
                            )
                    for h, oo in ((0, o0), (1, o1)):
                        nc.tensor.matmul(
                            oo[0:65, q0:QW],
                            vaug[:, 130 * j + 65 * h:130 * j + 65 * (h + 1)],
                            p[:, QW * h + q0:QW * (h + 1)],
                            start=(j == 0), stop=(j == nj - 1),
                        )
                return o0, o1

            def norm_send(t, o0, o1):
                # l rows (row 64 of o0/o1) -> SBUF (bf16), broadcast across
                # partitions via K=1 bf16 matmuls into a recycled s-tile,
                # reciprocal, then scale O^T and stage for the AllToAll.
                lrow = spool.tile([1, 2 * QW], BF16, tag="lrow")
                nc.vector.tensor_copy(lrow[0:1, 0:QW], o0[64:65, :])
                nc.vector.tensor_copy(lrow[0:1, QW:2 * QW], o1[64:65, :])
                lb = ps_s.tile([128, 2 * QW], F32, tag="s")
                for h in range(2):
                    nc.tensor.matmul(
                        lb[0:64, QW * h:QW * (h + 1)],
                        ones_sb[0:1, :],
                        lrow[0:1, QW * h:QW * (h + 1)],
                        start=True, stop=True,
                    )
                linv = spool.tile([64, 2 * QW], F32, tag="linv")
                nc.vector.reciprocal_approx_fast(linv[:], lb[0:64, :])
                ocn0 = spool.tile([64, QW], BF16, tag="ocn0")
                ocn1 = spool.tile([64, QW], BF16, tag="ocn1")
                nc.vector.tensor_mul(ocn0[:], o0[0:64, :], linv[:, 0:QW])
                nc.vector.tensor_mul(ocn1[:], o1[0:64, :], linv[:, QW:2 * QW])
                gi = group_of[t]
                base = GROUPS[gi].index(t) * TILE_FLAT
                for dims0, ocn in ((0, ocn0), (64 * 64, ocn1)):
                    nc.sync.dma_start(
                        parts[gi][:, base + dims0:base + dims0 + 64 * 64]
                        .rearrange("r (p c) -> p r c", p=64),
                        ocn[:].rearrange("p (r c) -> p r c", r=8),
                    )

            def launch_a2a(gi):
                nc.gpsimd.collective_compute(
                    "AllToAll",
                    mybir.AluOpType.bypass,
                    replica_groups=[[0, 1, 2, 3, 4, 5, 6, 7]],
                    ins=[parts[gi][:, :]],
                    outs=[recs[gi][:, :]],
                )

            def recv_oproj(t):
                # ysb cols 128g+[0:64] = batch-0 sender g, +[64:128] = batch-1
                # sender g: each 128-col block is one lhsT (same Wo rows).
                gi = group_of[t]
                base = GROUPS[gi].index(t) * TILE_FLAT
                ysb = stpool.tile([128, D], BF16, tag="ysb")
                for b in range(2):
                    nc.sync.dma_start(
                        ysb[:].rearrange("p (g b c) -> b p g c", g=4, b=2)[b],
                        recs[gi][4 * b:4 * (b + 1), base:base + TILE_FLAT]
                        .rearrange("g (p c) -> p g c", p=128),
                    )
                po = ps_p.tile([128, D], F32, tag="pr")
                for g in range(4):
                    nc.tensor.matmul(
                        po[:],
                        ysb[:, 128 * g:128 * (g + 1)],
                        wo_sb[:, D * g:D * (g + 1)],
                        start=(g == 0), stop=(g == 3),
                    )
                ost = stpool.tile([128, D], F32, tag="ost")
                nc.vector.tensor_add(ost[:], po[:], bob_sb[:])
                nc.sync.dma_start(out_ext[128 * t:128 * (t + 1), :], ost[:])

            proj(0)
            for t in range(NT):
                o0, o1 = att_jloop(t)
                norm_send(t, o0, o1)
                if t in (4, 6, 7):
                    launch_a2a(group_of[t])
                if t + 1 < NT:
                    proj(t + 1)
                if t == 6:
                    for tr in GROUPS[0]:
                        recv_oproj(tr)
            for tr in GROUPS[1] + GROUPS[2]:
                recv_oproj(tr)

    nc.finalize()
    return nc


def _make_in_maps(x, Wqkv, bqkv, Wo, bo):
    # causal 0/1 multiplicative mask for the diagonal 128x128 sub-block:
    # keep (p, o) where o >= p (k = block_base + p, q = block_base + o)
    p_idx = np.arange(128)[:, None]
    o_idx = np.arange(128)[None, :]
    maskt = (o_idx >= p_idx).astype(np.float32).astype(BF16_NP)

    in_maps = []
    for core in range(NCORES):
        b = core // 4
        g = core % 4
        rows = slice(128 * g, 128 * (g + 1))
        wq = Wqkv[0:D][rows]            # [128, 512]
        wk = Wqkv[D:2 * D][rows]
        wv = Wqkv[2 * D:3 * D][rows]
        wvT = np.zeros((D, 130), dtype=np.float32)
        wvT[:, 0:64] = wv[0:64].T
        wvT[:, 65:129] = wv[64:128].T
        bvb = np.zeros((128, 130), dtype=np.float32)
        bvb[:, 0:64] = bqkv[2 * D:3 * D][rows][0:64][None, :]
        bvb[:, 64] = 1.0
        bvb[:, 65:129] = bqkv[2 * D:3 * D][rows][64:128][None, :]
        bvb[:, 129] = 1.0
        in_maps.append({
            "xT": np.ascontiguousarray(x[b].T).astype(BF16_NP),
            "wqT": np.ascontiguousarray(wq.T).astype(BF16_NP),
            "wkT": np.ascontiguousarray(wk.T).astype(BF16_NP),
            "wvT": wvT.astype(BF16_NP),
            "bq": np.ascontiguousarray(bqkv[0:D][rows][:, None]).astype(np.float32),
            "bk": np.ascontiguousarray(bqkv[D:2 * D][rows][:, None]).astype(np.float32),
            "bvb": bvb,
            "woT": np.ascontiguousarray(Wo.T).astype(BF16_NP),
            "bob": np.tile(bo.astype(np.float32)[None, :], (128, 1)),
            "maskt": maskt,
            "ones": np.ones((1, HD), dtype=BF16_NP),
        })
    return in_maps


def run(x, Wqkv, bqkv, Wo, bo, trace=False):
    if "nc" not in _CACHE:
        _CACHE["nc"] = _build_nc()
    nc = _CACHE["nc"]
    in_maps = _make_in_maps(x, Wqkv, bqkv, Wo, bo)
    res = run_bass_kernel_spmd(nc, in_maps, core_ids=list(range(NCORES)), trace=trace)
    out = np.empty((B, S, D), dtype=np.float32)
    for core in range(NCORES):
        o = res.results[core]["out"]
        # 8-way AllToAll: core j owns tile-t tokens 512t+64j..+64 for BOTH
        # batches (rows 128t..+64 = batch 0, rows 128t+64..+128 = batch 1)
        for t in range(NT):
            tok = QW * t + 64 * core
            out[0, tok:tok + 64, :] = o[128 * t:128 * t + 64]
            out[1, tok:tok + 64, :] = o[128 * t + 64:128 * (t + 1)]
    return out, res


def kernel(x, Wqkv, bqkv, Wo, bo):
    out, _ = run(np.asarray(x, dtype=np.float32), np.asarray(Wqkv, dtype=np.float32),
                 np.asarray(bqkv, dtype=np.float32), np.asarray(Wo, dtype=np.float32),
                 np.asarray(bo, dtype=np.float32))
    return out


# revision 14
# speedup vs baseline: 1.0741x; 1.0520x over previous
"""Distributed causal-attention block (dense_transformer) on 8 TRN2 NeuronCores.

Sharding: data-parallel over batch (b=2) x tensor-parallel over head pairs
(8 heads -> 4 groups of 2). Core i handles batch i//4, heads (2*(i%4), 2*(i%4)+1).

Per-core pipeline (software-pipelined across the 8 q tiles):
  - token-chunked QKV projections (transposed layouts; V natural+ones column)
  - block-causal flash-style attention (S^T = K @ Q^T, denominator via the
    augmented ones-column in V); exp on the scalar engine, diagonal causal
    masks on the (otherwise idle) gpsimd engine
  - per-tile softmax normalization on the sender: l broadcast across
    partitions via a K=1 fp32 matmul into a recycled PSUM tile, fast
    reciprocal, one tensor_mul per head
  - AllToAll (4x less wire traffic than ReduceScatter of partial O-proj
    sums) redistributes normalized head outputs so each core owns a token
    quarter with all 512 head dims, then a local O projection + bias.

B, S, D, H = 2, 4096, 512, 8 (hd=64). Hardcoded per problem spec.
"""

import numpy as np
import ml_dtypes

import concourse.bacc as bacc
import concourse.mybir as mybir
from concourse import tile
from concourse.bass_utils import run_bass_kernel_spmd

B, S, D = 2, 4096, 512
H = 8
HD = D // H          # 64
NCORES = 8
R = 128              # qkv rows per core (2 heads x 64)
NT = 8               # q tiles of 512
QW = 512             # q tile width

BF16 = mybir.dt.bfloat16
F32 = mybir.dt.float32
AF = mybir.ActivationFunctionType
BF16_NP = ml_dtypes.bfloat16

_CACHE = {}


def _build_nc():
    nc = bacc.Bacc(num_devices=NCORES)

    xT = nc.declare_dram_parameter("xT", [D, S], BF16, isOutput=False)
    wqT = nc.declare_dram_parameter("wqT", [D, R], BF16, isOutput=False)
    wkT = nc.declare_dram_parameter("wkT", [D, R], BF16, isOutput=False)
    wvT = nc.declare_dram_parameter("wvT", [D, 130], BF16, isOutput=False)
    bq = nc.declare_dram_parameter("bq", [R, 1], F32, isOutput=False)
    bk = nc.declare_dram_parameter("bk", [R, 1], F32, isOutput=False)
    bvb = nc.declare_dram_parameter("bvb", [128, 130], F32, isOutput=False)
    woT = nc.declare_dram_parameter("woT", [D, D], BF16, isOutput=False)
    bob = nc.declare_dram_parameter("bob", [128, D], F32, isOutput=False)
    maskt = nc.declare_dram_parameter("maskt", [128, 128], BF16, isOutput=False)
    ones = nc.declare_dram_parameter("ones", [1, HD], BF16, isOutput=False)
    out_ext = nc.declare_dram_parameter("out", [NT * 128, D], F32, isOutput=True)

    # AllToAll payload (one per q tile; ~15us each, pipelined 4 tiles deep
    # behind compute). Chunk j = this core's 128 head-dims for tile tokens
    # 64j..64(j+1), flattened [p, c] -> p*64+c; rec row i = the same from
    # global rank i (ranks 0-3 batch 0, 4-7 batch 1).
    GROUPS = [(t,) for t in range(NT)]
    TILE_FLAT = 128 * 64
    group_of = {t: gi for gi, g in enumerate(GROUPS) for t in g}
    parts = [nc.dram_tensor(f"part{gi}", [8, len(g) * TILE_FLAT], BF16)
             for gi, g in enumerate(GROUPS)]
    recs = [nc.dram_tensor(f"rec{gi}", [8, len(g) * TILE_FLAT], BF16)
            for gi, g in enumerate(GROUPS)]

    with tile.TileContext(nc) as tc:
        with (
            tc.tile_pool(name="const", bufs=1) as cpool,
            tc.tile_pool(name="xres", bufs=1) as xpool,
            tc.tile_pool(name="pt", bufs=12) as ppool,
            tc.tile_pool(name="small", bufs=3) as spool,
            tc.tile_pool(name="stage", bufs=3) as stpool,
            tc.tile_pool(name="ps_s", bufs=2, space="PSUM") as ps_s,
            tc.tile_pool(name="ps_o", bufs=1, space="PSUM") as ps_o,
            tc.tile_pool(name="ps_p", bufs=1, space="PSUM") as ps_p,
        ):
            # ---------- weights / constants into SBUF (attention-critical first)
            wq_sb = cpool.tile([128, D], BF16)
            nc.sync.dma_start(wq_sb[:].rearrange("p (c m) -> p c m", c=4),
                              wqT[:, :].rearrange("(c p) m -> p c m", p=128))
            wk_sb = cpool.tile([128, D], BF16)
            nc.sync.dma_start(wk_sb[:].rearrange("p (c m) -> p c m", c=4),
                              wkT[:, :].rearrange("(c p) m -> p c m", p=128))
            wv_sb = cpool.tile([128, 4 * 130], BF16)
            nc.sync.dma_start(wv_sb[:].rearrange("p (c m) -> p c m", c=4),
                              wvT[:, :].rearrange("(c p) m -> p c m", p=128))
            bq_sb = cpool.tile([R, 1], F32)
            nc.sync.dma_start(bq_sb[:], bq[:, :])
            bk_sb = cpool.tile([R, 1], F32)
            nc.sync.dma_start(bk_sb[:], bk[:, :])
            bvb_sb = cpool.tile([128, 130], F32)
            nc.sync.dma_start(bvb_sb[:], bvb[:, :])
            mask_sb = cpool.tile([128, 128], BF16)
            nc.sync.dma_start(mask_sb[:], maskt[:, :])
            ones_sb = cpool.tile([1, HD], BF16)
            nc.sync.dma_start(ones_sb[:], ones[:, :])

            xt = [xpool.tile([128, S], BF16, tag=f"xt{c}", name=f"xt{c}")
                  for c in range(4)]
            qT = xpool.tile([128, S], BF16, tag="qT")
            kT = xpool.tile([128, S], BF16, tag="kT")
            vaug = xpool.tile([128, 32 * 130], BF16, tag="vaug")

            def load_x(nt):
                for c in range(4):
                    nc.sync.dma_start(
                        xt[c][:, QW * nt:QW * (nt + 1)],
                        xT[128 * c:128 * (c + 1), QW * nt:QW * (nt + 1)],
                    )

            load_x(0)
            load_x(1)
            # O-proj weights needed only from the first epilogue onward
            wo_sb = cpool.tile([128, 4 * D], BF16)
            nc.sync.dma_start(wo_sb[:].rearrange("p (g m) -> p g m", g=4),
                              woT[:, :].rearrange("(g p) m -> p g m", p=128))
            bob_sb = cpool.tile([128, D], F32)
            nc.sync.dma_start(bob_sb[:], bob[:, :])
            for nt in range(2, NT):
                load_x(nt)

            def proj(nt):
                # Q^T / K^T rows for token block nt
                for w_sb, b_sb, dst in ((wq_sb, bq_sb, qT), (wk_sb, bk_sb, kT)):
                    ps = ps_p.tile([128, QW], F32, tag="po")
                    for c in range(4):
                        nc.tensor.matmul(
                            ps[:],
                            w_sb[:, 128 * c:128 * (c + 1)],
                            xt[c][:, QW * nt:QW * (nt + 1)],
                            start=(c == 0), stop=(c == 3),
                        )
                    nc.vector.tensor_scalar_add(
                        dst[:, QW * nt:QW * (nt + 1)], ps[:], b_sb[:])
                # V natural (augmented with ones col per head)
                for tb in range(4 * nt, 4 * (nt + 1)):
                    ps = ps_p.tile([128, QW], F32, tag="po")
                    for c in range(4):
                        nc.tensor.matmul(
                            ps[:, 0:130],
                            xt[c][:, 128 * tb:128 * (tb + 1)],
                            wv_sb[:, 130 * c:130 * (c + 1)],
                            start=(c == 0), stop=(c == 3),
                        )
                    nc.vector.tensor_add(
                        vaug[:, 130 * tb:130 * (tb + 1)], ps[:, 0:130], bvb_sb[:]
                    )

            def att_jloop(t):
                nj = 4 * t + 4          # causal: k blocks 0 .. 4t+3
                o0 = ps_o.tile([128, QW], F32, tag="o0")
                o1 = ps_o.tile([128, QW], F32, tag="o1")
                for j in range(nj):
                    # causal: q columns < q0 are fully masked for this k block
                    q0 = max(0, 128 * (j - 4 * t))
                    s = ps_s.tile([128, 2 * QW], F32, tag="s")
                    for h in (0, 1):
                        nc.tensor.matmul(
                            s[:, QW * h + q0:QW * (h + 1)],
                            kT[64 * h:64 * (h + 1), 128 * j:128 * (j + 1)],
                            qT[64 * h:64 * (h + 1), QW * t + q0:QW * (t + 1)],
                            start=True, stop=True,
                        )
                    p = ppool.tile([128, 2 * QW], BF16, tag="p")
                    if q0 == 0:
                        nc.scalar.activation(p[:], s[:], AF.Exp, bias=0.0, scale=0.125)
                    else:
                        sv = s[:].rearrange("k (h q) -> k h q", h=2)[:, :, q0:QW]
                        pv = p[:].rearrange("k (h q) -> k h q", h=2)[:, :, q0:QW]
                        nc.scalar.activation(pv, sv, AF.Exp, bias=0.0, scale=0.125)
                    if j >= 4 * t:  # diagonal 128-col boundary: 0/1 mask
                        for h in (0, 1):
                            nc.gpsimd.tensor_mul(
                                p[:, QW * h + q0:QW * h + q0 + 128],
                                p[:, QW * h + q0:QW * h + q0 + 128],
                                mask_sb[:, :],
                            )
                    for h, oo in ((0, o0), (1, o1)):
                        nc.tensor.matmul(
                            oo[0:65, q0:QW],
                            vaug[:, 130 * j + 65 * h:130 * j + 65 * (h + 1)],
                            p[:, QW * h + q0:QW * (h + 1)],
                            start=(j == 0), stop=(j == nj - 1),
                        )
                return o0, o1

            def norm_send(t, o0, o1):
                # l rows (row 64 of o0/o1) -> SBUF (bf16), broadcast across
                # partitions via K=1 bf16 matmuls into a recycled s-tile,
                # reciprocal, then scale O^T and stage for the AllToAll.
                lrow = spool.tile([1, 2 * QW], BF16, tag="lrow")
                nc.vector.tensor_copy(lrow[0:1, 0:QW], o0[64:65, :])
                nc.vector.tensor_copy(lrow[0:1, QW:2 * QW], o1[64:65, :])
                lb = ps_s.tile([128, 2 * QW], F32, tag="s")
                for h in range(2):
                    nc.tensor.matmul(
                        lb[0:64, QW * h:QW * (h + 1)],
                        ones_sb[0:1, :],
                        lrow[0:1, QW * h:QW * (h + 1)],
                        start=True, stop=True,
                    )
                linv = spool.tile([64, 2 * QW], F32, tag="linv")
                nc.vector.reciprocal_approx_fast(linv[:], lb[0:64, :])
                ocn0 = spool.tile([64, QW], BF16, tag="ocn0")
                ocn1 = spool.tile([64, QW], BF16, tag="ocn1")
                nc.vector.tensor_mul(ocn0[:], o0[0:64, :], linv[:, 0:QW])
                nc.vector.tensor_mul(ocn1[:], o1[0:64, :], linv[:, QW:2 * QW])
                gi = group_of[t]
                base = GROUPS[gi].index(t) * TILE_FLAT
                for dims0, ocn in ((0, ocn0), (64 * 64, ocn1)):
                    nc.sync.dma_start(
                        parts[gi][:, base + dims0:base + dims0 + 64 * 64]
                        .rearrange("r (p c) -> p r c", p=64),
                        ocn[:].rearrange("p (r c) -> p r c", r=8),
                    )

            def launch_a2a(gi):
                nc.gpsimd.collective_compute(
                    "AllToAll",
                    mybir.AluOpType.bypass,
                    replica_groups=[[0, 1, 2, 3, 4, 5, 6, 7]],
                    ins=[parts[gi][:, :]],
                    outs=[recs[gi][:, :]],
                )

            def recv_oproj(t):
                # ysb cols 128g+[0:64] = batch-0 sender g, +[64:128] = batch-1
                # sender g: each 128-col block is one lhsT (same Wo rows).
                gi = group_of[t]
                base = GROUPS[gi].index(t) * TILE_FLAT
                ysb = stpool.tile([128, D], BF16, tag="ysb")
                for b in range(2):
                    nc.sync.dma_start(
                        ysb[:].rearrange("p (g b c) -> b p g c", g=4, b=2)[b],
                        recs[gi][4 * b:4 * (b + 1), base:base + TILE_FLAT]
                        .rearrange("g (p c) -> p g c", p=128),
                    )
                po = ps_p.tile([128, D], F32, tag="pr")
                for g in range(4):
                    nc.tensor.matmul(
                        po[:],
                        ysb[:, 128 * g:128 * (g + 1)],
                        wo_sb[:, D * g:D * (g + 1)],
                        start=(g == 0), stop=(g == 3),
                    )
                ost = stpool.tile([128, D], F32, tag="ost")
                nc.vector.tensor_add(ost[:], po[:], bob_sb[:])
                nc.sync.dma_start(out_ext[128 * t:128 * (t + 1), :], ost[:])

            proj(0)
            for t in range(NT):
                o0, o1 = att_jloop(t)
                norm_send(t, o0, o1)
                launch_a2a(group_of[t])
                if t + 1 < NT:
                    proj(t + 1)
                if t >= 4:
                    recv_oproj(t - 4)
            for tr in range(NT - 4, NT):
                recv_oproj(tr)

    nc.finalize()
    return nc


def _make_in_maps(x, Wqkv, bqkv, Wo, bo):
    # causal 0/1 multiplicative mask for the diagonal 128x128 sub-block:
    # keep (p, o) where o >= p (k = block_base + p, q = block_base + o)
    p_idx = np.arange(128)[:, None]
    o_idx = np.arange(128)[None, :]
    maskt = (o_idx >= p_idx).astype(np.float32).astype(BF16_NP)

    in_maps = []
    for core in range(NCORES):
        b = core // 4
        g = core % 4
        rows = slice(128 * g, 128 * (g + 1))
        wq = Wqkv[0:D][rows]            # [128, 512]
        wk = Wqkv[D:2 * D][rows]
        wv = Wqkv[2 * D:3 * D][rows]
        wvT = np.zeros((D, 130), dtype=np.float32)
        wvT[:, 0:64] = wv[0:64].T
        wvT[:, 65:129] = wv[64:128].T
        bvb = np.zeros((128, 130), dtype=np.float32)
        bvb[:, 0:64] = bqkv[2 * D:3 * D][rows][0:64][None, :]
        bvb[:, 64] = 1.0
        bvb[:, 65:129] = bqkv[2 * D:3 * D][rows][64:128][None, :]
        bvb[:, 129] = 1.0
        in_maps.append({
            "xT": np.ascontiguousarray(x[b].T).astype(BF16_NP),
            "wqT": np.ascontiguousarray(wq.T).astype(BF16_NP),
            "wkT": np.ascontiguousarray(wk.T).astype(BF16_NP),
            "wvT": wvT.astype(BF16_NP),
            "bq": np.ascontiguousarray(bqkv[0:D][rows][:, None]).astype(np.float32),
            "bk": np.ascontiguousarray(bqkv[D:2 * D][rows][:, None]).astype(np.float32),
            "bvb": bvb,
            "woT": np.ascontiguousarray(Wo.T).astype(BF16_NP),
            "bob": np.tile(bo.astype(np.float32)[None, :], (128, 1)),
            "maskt": maskt,
            "ones": np.ones((1, HD), dtype=BF16_NP),
        })
    return in_maps


def run(x, Wqkv, bqkv, Wo, bo, trace=False):
    if "nc" not in _CACHE:
        _CACHE["nc"] = _build_nc()
    nc = _CACHE["nc"]
    in_maps = _make_in_maps(x, Wqkv, bqkv, Wo, bo)
    res = run_bass_kernel_spmd(nc, in_maps, core_ids=list(range(NCORES)), trace=trace)
    out = np.empty((B, S, D), dtype=np.float32)
    for core in range(NCORES):
        o = res.results[core]["out"]
        # 8-way AllToAll: core j owns tile-t tokens 512t+64j..+64 for BOTH
        # batches (rows 128t..+64 = batch 0, rows 128t+64..+128 = batch 1)
        for t in range(NT):
            tok = QW * t + 64 * core
            out[0, tok:tok + 64, :] = o[128 * t:128 * t + 64]
            out[1, tok:tok + 64, :] = o[128 * t + 64:128 * (t + 1)]
    return out, res


def kernel(x, Wqkv, bqkv, Wo, bo):
    out, _ = run(np.asarray(x, dtype=np.float32), np.asarray(Wqkv, dtype=np.float32),
                 np.asarray(bqkv, dtype=np.float32), np.asarray(Wo, dtype=np.float32),
                 np.asarray(bo, dtype=np.float32))
    return out


# revision 17
# speedup vs baseline: 1.1951x; 1.1126x over previous
"""Distributed causal-attention block (dense_transformer) on 8 TRN2 NeuronCores.

Sharding: data-parallel over batch (b=2) x tensor-parallel over head pairs
(8 heads -> 4 groups of 2). Core i handles batch i//4, heads (2*(i%4), 2*(i%4)+1).

Per-core pipeline (software-pipelined across the 8 q tiles):
  - token-chunked QKV projections (transposed layouts; V natural+ones column)
  - block-causal flash-style attention (S^T = K @ Q^T, denominator via the
    augmented ones-column in V); exp on the scalar engine, diagonal causal
    masks on the (otherwise idle) gpsimd engine
  - per-tile softmax normalization on the sender: l broadcast across
    partitions via a K=1 fp32 matmul into a recycled PSUM tile, fast
    reciprocal, one tensor_mul per head
  - AllToAll (4x less wire traffic than ReduceScatter of partial O-proj
    sums) redistributes normalized head outputs so each core owns a token
    quarter with all 512 head dims, then a local O projection + bias.

B, S, D, H = 2, 4096, 512, 8 (hd=64). Hardcoded per problem spec.
"""

import numpy as np
import ml_dtypes

import concourse.bacc as bacc
import concourse.mybir as mybir
from concourse import tile
from concourse.bass_utils import run_bass_kernel_spmd

B, S, D = 2, 4096, 512
H = 8
HD = D // H          # 64
NCORES = 8
R = 128              # qkv rows per core (2 heads x 64)
NT = 8               # q tiles of 512
QW = 512             # q tile width

BF16 = mybir.dt.bfloat16
F32 = mybir.dt.float32
AF = mybir.ActivationFunctionType
BF16_NP = ml_dtypes.bfloat16

_CACHE = {}


def _build_nc():
    nc = bacc.Bacc(num_devices=NCORES)

    xT = nc.declare_dram_parameter("xT", [D, S], BF16, isOutput=False)
    wqT = nc.declare_dram_parameter("wqT", [D, R], BF16, isOutput=False)
    wkT = nc.declare_dram_parameter("wkT", [D, R], BF16, isOutput=False)
    wvT = nc.declare_dram_parameter("wvT", [D, 130], BF16, isOutput=False)
    bq = nc.declare_dram_parameter("bq", [R, 1], F32, isOutput=False)
    bk = nc.declare_dram_parameter("bk", [R, 1], F32, isOutput=False)
    bvb = nc.declare_dram_parameter("bvb", [128, 130], F32, isOutput=False)
    woT = nc.declare_dram_parameter("woT", [D, D], BF16, isOutput=False)
    bob = nc.declare_dram_parameter("bob", [128, D], F32, isOutput=False)
    maskt = nc.declare_dram_parameter("maskt", [128, 128], BF16, isOutput=False)
    ones = nc.declare_dram_parameter("ones", [1, HD], BF16, isOutput=False)
    out_ext = nc.declare_dram_parameter("out", [NT * 128, D], F32, isOutput=True)

    # AllToAll payload (one per q tile; ~15us each, pipelined 4 tiles deep
    # behind compute). Chunk j = this core's 128 head-dims for tile tokens
    # 64j..64(j+1), flattened [p, c] -> p*64+c; rec row i = the same from
    # global rank i (ranks 0-3 batch 0, 4-7 batch 1).
    GROUPS = [(t,) for t in range(NT)]
    TILE_FLAT = 128 * 64
    group_of = {t: gi for gi, g in enumerate(GROUPS) for t in g}
    parts = [nc.dram_tensor(f"part{gi}", [8, len(g) * TILE_FLAT], BF16)
             for gi, g in enumerate(GROUPS)]
    recs = [nc.dram_tensor(f"rec{gi}", [8, len(g) * TILE_FLAT], BF16)
            for gi, g in enumerate(GROUPS)]

    with tile.TileContext(nc) as tc:
        with (
            tc.tile_pool(name="const", bufs=1) as cpool,
            tc.tile_pool(name="xres", bufs=1) as xpool,
            tc.tile_pool(name="pt", bufs=12) as ppool,
            tc.tile_pool(name="small", bufs=3) as spool,
            tc.tile_pool(name="stage", bufs=3) as stpool,
            tc.tile_pool(name="ps_s", bufs=2, space="PSUM") as ps_s,
            tc.tile_pool(name="ps_o", bufs=1, space="PSUM") as ps_o,
            tc.tile_pool(name="ps_p", bufs=1, space="PSUM") as ps_p,
        ):
            # ---------- weights / constants into SBUF (attention-critical first)
            wq_sb = cpool.tile([128, D], BF16)
            nc.sync.dma_start(wq_sb[:].rearrange("p (c m) -> p c m", c=4),
                              wqT[:, :].rearrange("(c p) m -> p c m", p=128))
            wk_sb = cpool.tile([128, D], BF16)
            nc.sync.dma_start(wk_sb[:].rearrange("p (c m) -> p c m", c=4),
                              wkT[:, :].rearrange("(c p) m -> p c m", p=128))
            wv_sb = cpool.tile([128, 4 * 130], BF16)
            nc.sync.dma_start(wv_sb[:].rearrange("p (c m) -> p c m", c=4),
                              wvT[:, :].rearrange("(c p) m -> p c m", p=128))
            bq_sb = cpool.tile([R, 1], F32)
            nc.sync.dma_start(bq_sb[:], bq[:, :])
            bk_sb = cpool.tile([R, 1], F32)
            nc.sync.dma_start(bk_sb[:], bk[:, :])
            bvb_sb = cpool.tile([128, 130], F32)
            nc.sync.dma_start(bvb_sb[:], bvb[:, :])
            mask_sb = cpool.tile([128, 128], BF16)
            nc.sync.dma_start(mask_sb[:], maskt[:, :])
            ones_sb = cpool.tile([1, HD], BF16)
            nc.sync.dma_start(ones_sb[:], ones[:, :])

            xt = [xpool.tile([128, S], BF16, tag=f"xt{c}", name=f"xt{c}")
                  for c in range(4)]
            qT = xpool.tile([128, S], BF16, tag="qT")
            kT = xpool.tile([128, S], BF16, tag="kT")
            vaug = xpool.tile([128, 32 * 130], BF16, tag="vaug")

            def load_x(c, half):
                nc.sync.dma_start(
                    xt[c][:, (S // 2) * half:(S // 2) * (half + 1)],
                    xT[128 * c:128 * (c + 1),
                       (S // 2) * half:(S // 2) * (half + 1)],
                )

            for c in range(4):
                load_x(c, 0)
            # O-proj weights needed only from the first epilogue onward
            wo_sb = cpool.tile([128, 4 * D], BF16)
            nc.sync.dma_start(wo_sb[:].rearrange("p (g m) -> p g m", g=4),
                              woT[:, :].rearrange("(g p) m -> p g m", p=128))
            bob_sb = cpool.tile([128, D], F32)
            nc.sync.dma_start(bob_sb[:], bob[:, :])
            for c in range(4):
                load_x(c, 1)

            def proj_units(nt):
                # 6 small work units (q, k, 4x v) for token block nt, fed
                # into the j-loop slack slot so exp never starves.
                def qk_unit(w_sb, b_sb, dst):
                    def u():
                        ps = ps_p.tile([128, QW], F32, tag="po", name="ps_qk")
                        for c in range(4):
                            nc.tensor.matmul(
                                ps[:],
                                w_sb[:, 128 * c:128 * (c + 1)],
                                xt[c][:, QW * nt:QW * (nt + 1)],
                                start=(c == 0), stop=(c == 3),
                            )
                        nc.vector.tensor_scalar_add(
                            dst[:, QW * nt:QW * (nt + 1)], ps[:], b_sb[:])
                    return u

                def v_unit(tb):
                    def u():
                        ps = ps_p.tile([128, QW], F32, tag="po", name="ps_v")
                        for c in range(4):
                            nc.tensor.matmul(
                                ps[:, 0:130],
                                xt[c][:, 128 * tb:128 * (tb + 1)],
                                wv_sb[:, 130 * c:130 * (c + 1)],
                                start=(c == 0), stop=(c == 3),
                            )
                        nc.vector.tensor_add(
                            vaug[:, 130 * tb:130 * (tb + 1)], ps[:, 0:130],
                            bvb_sb[:])
                    return u

                return ([qk_unit(wq_sb, bq_sb, qT), qk_unit(wk_sb, bk_sb, kT)]
                        + [v_unit(tb) for tb in range(4 * nt, 4 * (nt + 1))])

            def att_jloop(t, units):
                # `units`: deferred work (next tile's projections, old tiles'
                # O-proj) emitted between the s matmul and PV of a j
                # iteration — the tensor engine fills that slot while the
                # scalar engine runs exp, so exp never starves at tile
                # boundaries.
                nj = 4 * t + 4          # causal: k blocks 0 .. 4t+3
                o0 = ps_o.tile([128, QW], F32, tag="o0")
                o1 = ps_o.tile([128, QW], F32, tag="o1")
                n_units = len(units)
                emitted = 0
                for j in range(nj):
                    # causal: q columns < q0 are fully masked for this k block
                    q0 = max(0, 128 * (j - 4 * t))
                    s = ps_s.tile([128, 2 * QW], F32, tag="s")
                    for h in (0, 1):
                        nc.tensor.matmul(
                            s[:, QW * h + q0:QW * (h + 1)],
                            kT[64 * h:64 * (h + 1), 128 * j:128 * (j + 1)],
                            qT[64 * h:64 * (h + 1), QW * t + q0:QW * (t + 1)],
                            start=True, stop=True,
                        )
                    p = ppool.tile([128, 2 * QW], BF16, tag="p")
                    if q0 == 0:
                        nc.scalar.activation(p[:], s[:], AF.Exp, bias=0.0, scale=0.125)
                    else:
                        sv = s[:].rearrange("k (h q) -> k h q", h=2)[:, :, q0:QW]
                        pv = p[:].rearrange("k (h q) -> k h q", h=2)[:, :, q0:QW]
                        nc.scalar.activation(pv, sv, AF.Exp, bias=0.0, scale=0.125)
                    while emitted < n_units and emitted <= j * n_units // nj:
                        units[emitted]()
                        emitted += 1
                    if j >= 4 * t:  # diagonal 128-col boundary: 0/1 mask
                        for h in (0, 1):
                            nc.gpsimd.tensor_mul(
                                p[:, QW * h + q0:QW * h + q0 + 128],
                                p[:, QW * h + q0:QW * h + q0 + 128],
                                mask_sb[:, :],
                            )
                    for h, oo in ((0, o0), (1, o1)):
                        nc.tensor.matmul(
                            oo[0:65, q0:QW],
                            vaug[:, 130 * j + 65 * h:130 * j + 65 * (h + 1)],
                            p[:, QW * h + q0:QW * (h + 1)],
                            start=(j == 0), stop=(j == nj - 1),
                        )
                for u in units[emitted:]:   # leftovers (defensive)
                    u()
                return o0, o1

            def norm_send(t, o0, o1):
                # l rows (row 64 of o0/o1) -> SBUF (bf16), broadcast across
                # partitions via K=1 bf16 matmuls into a recycled s-tile,
                # reciprocal, then scale O^T and stage for the AllToAll.
                lrow = spool.tile([1, 2 * QW], BF16, tag="lrow")
                nc.vector.tensor_copy(lrow[0:1, 0:QW], o0[64:65, :])
                nc.vector.tensor_copy(lrow[0:1, QW:2 * QW], o1[64:65, :])
                lb = ps_s.tile([128, 2 * QW], F32, tag="s")
                for h in range(2):
                    nc.tensor.matmul(
                        lb[0:64, QW * h:QW * (h + 1)],
                        ones_sb[0:1, :],
                        lrow[0:1, QW * h:QW * (h + 1)],
                        start=True, stop=True,
                    )
                linv = spool.tile([64, 2 * QW], F32, tag="linv")
                nc.vector.reciprocal_approx_fast(linv[:], lb[0:64, :])
                ocn0 = spool.tile([64, QW], BF16, tag="ocn0")
                ocn1 = spool.tile([64, QW], BF16, tag="ocn1")
                nc.vector.tensor_mul(ocn0[:], o0[0:64, :], linv[:, 0:QW])
                nc.vector.tensor_mul(ocn1[:], o1[0:64, :], linv[:, QW:2 * QW])
                gi = group_of[t]
                base = GROUPS[gi].index(t) * TILE_FLAT
                for dims0, ocn in ((0, ocn0), (64 * 64, ocn1)):
                    nc.sync.dma_start(
                        parts[gi][:, base + dims0:base + dims0 + 64 * 64]
                        .rearrange("r (p c) -> p r c", p=64),
                        ocn[:].rearrange("p (r c) -> p r c", r=8),
                    )

            def launch_a2a(gi):
                nc.gpsimd.collective_compute(
                    "AllToAll",
                    mybir.AluOpType.bypass,
                    replica_groups=[[0, 1, 2, 3, 4, 5, 6, 7]],
                    ins=[parts[gi][:, :]],
                    outs=[recs[gi][:, :]],
                )

            def recv_oproj(t):
                # ysb cols 128g+[0:64] = batch-0 sender g, +[64:128] = batch-1
                # sender g: each 128-col block is one lhsT (same Wo rows).
                gi = group_of[t]
                base = GROUPS[gi].index(t) * TILE_FLAT
                ysb = stpool.tile([128, D], BF16, tag="ysb")
                for b in range(2):
                    nc.sync.dma_start(
                        ysb[:].rearrange("p (g b c) -> b p g c", g=4, b=2)[b],
                        recs[gi][4 * b:4 * (b + 1), base:base + TILE_FLAT]
                        .rearrange("g (p c) -> p g c", p=128),
                    )
                po = ps_p.tile([128, D], F32, tag="pr")
                for g in range(4):
                    nc.tensor.matmul(
                        po[:],
                        ysb[:, 128 * g:128 * (g + 1)],
                        wo_sb[:, D * g:D * (g + 1)],
                        start=(g == 0), stop=(g == 3),
                    )
                ost = stpool.tile([128, D], F32, tag="ost")
                nc.vector.tensor_add(ost[:], po[:], bob_sb[:])
                nc.sync.dma_start(out_ext[128 * t:128 * (t + 1), :], ost[:])

            for u in proj_units(0):
                u()
            for t in range(NT):
                units = []
                if t >= 4:
                    units.append(lambda tr=t - 4: recv_oproj(tr))
                if t + 1 < NT:
                    units.extend(proj_units(t + 1))
                o0, o1 = att_jloop(t, units)
                norm_send(t, o0, o1)
                launch_a2a(group_of[t])
            for tr in range(NT - 4, NT):
                recv_oproj(tr)

    nc.finalize()
    return nc


def _make_in_maps(x, Wqkv, bqkv, Wo, bo):
    # causal 0/1 multiplicative mask for the diagonal 128x128 sub-block:
    # keep (p, o) where o >= p (k = block_base + p, q = block_base + o)
    p_idx = np.arange(128)[:, None]
    o_idx = np.arange(128)[None, :]
    maskt = (o_idx >= p_idx).astype(np.float32).astype(BF16_NP)

    in_maps = []
    for core in range(NCORES):
        b = core // 4
        g = core % 4
        rows = slice(128 * g, 128 * (g + 1))
        wq = Wqkv[0:D][rows]            # [128, 512]
        wk = Wqkv[D:2 * D][rows]
        wv = Wqkv[2 * D:3 * D][rows]
        wvT = np.zeros((D, 130), dtype=np.float32)
        wvT[:, 0:64] = wv[0:64].T
        wvT[:, 65:129] = wv[64:128].T
        bvb = np.zeros((128, 130), dtype=np.float32)
        bvb[:, 0:64] = bqkv[2 * D:3 * D][rows][0:64][None, :]
        bvb[:, 64] = 1.0
        bvb[:, 65:129] = bqkv[2 * D:3 * D][rows][64:128][None, :]
        bvb[:, 129] = 1.0
        in_maps.append({
            "xT": np.ascontiguousarray(x[b].T).astype(BF16_NP),
            "wqT": np.ascontiguousarray(wq.T).astype(BF16_NP),
            "wkT": np.ascontiguousarray(wk.T).astype(BF16_NP),
            "wvT": wvT.astype(BF16_NP),
            "bq": np.ascontiguousarray(bqkv[0:D][rows][:, None]).astype(np.float32),
            "bk": np.ascontiguousarray(bqkv[D:2 * D][rows][:, None]).astype(np.float32),
            "bvb": bvb,
            "woT": np.ascontiguousarray(Wo.T).astype(BF16_NP),
            "bob": np.tile(bo.astype(np.float32)[None, :], (128, 1)),
            "maskt": maskt,
            "ones": np.ones((1, HD), dtype=BF16_NP),
        })
    return in_maps


def run(x, Wqkv, bqkv, Wo, bo, trace=False):
    if "nc" not in _CACHE:
        _CACHE["nc"] = _build_nc()
    nc = _CACHE["nc"]
    in_maps = _make_in_maps(x, Wqkv, bqkv, Wo, bo)
    res = run_bass_kernel_spmd(nc, in_maps, core_ids=list(range(NCORES)), trace=trace)
    out = np.empty((B, S, D), dtype=np.float32)
    for core in range(NCORES):
        o = res.results[core]["out"]
        # 8-way AllToAll: core j owns tile-t tokens 512t+64j..+64 for BOTH
        # batches (rows 128t..+64 = batch 0, rows 128t+64..+128 = batch 1)
        for t in range(NT):
            tok = QW * t + 64 * core
            out[0, tok:tok + 64, :] = o[128 * t:128 * t + 64]
            out[1, tok:tok + 64, :] = o[128 * t + 64:128 * (t + 1)]
    return out, res


def kernel(x, Wqkv, bqkv, Wo, bo):
    out, _ = run(np.asarray(x, dtype=np.float32), np.asarray(Wqkv, dtype=np.float32),
                 np.asarray(bqkv, dtype=np.float32), np.asarray(Wo, dtype=np.float32),
                 np.asarray(bo, dtype=np.float32))
    return out


# revision 18
# speedup vs baseline: 1.1964x; 1.0011x over previous
"""Distributed causal-attention block (dense_transformer) on 8 TRN2 NeuronCores.

Sharding: data-parallel over batch (b=2) x tensor-parallel over head pairs
(8 heads -> 4 groups of 2). Core i handles batch i//4, heads (2*(i%4), 2*(i%4)+1).

Per-core pipeline (software-pipelined across the 8 q tiles):
  - token-chunked QKV projections (transposed layouts; V natural+ones column)
  - block-causal flash-style attention (S^T = K @ Q^T, denominator via the
    augmented ones-column in V); exp on the scalar engine, diagonal causal
    masks on the (otherwise idle) gpsimd engine
  - per-tile softmax normalization on the sender: l broadcast across
    partitions via a K=1 fp32 matmul into a recycled PSUM tile, fast
    reciprocal, one tensor_mul per head
  - AllToAll (4x less wire traffic than ReduceScatter of partial O-proj
    sums) redistributes normalized head outputs so each core owns a token
    quarter with all 512 head dims, then a local O projection + bias.

B, S, D, H = 2, 4096, 512, 8 (hd=64). Hardcoded per problem spec.
"""

import numpy as np
import ml_dtypes

import concourse.bacc as bacc
import concourse.mybir as mybir
from concourse import tile
from concourse.bass_utils import run_bass_kernel_spmd

B, S, D = 2, 4096, 512
H = 8
HD = D // H          # 64
NCORES = 8
R = 128              # qkv rows per core (2 heads x 64)
NT = 8               # q tiles of 512
QW = 512             # q tile width

BF16 = mybir.dt.bfloat16
F32 = mybir.dt.float32
AF = mybir.ActivationFunctionType
BF16_NP = ml_dtypes.bfloat16

_CACHE = {}


def _build_nc():
    nc = bacc.Bacc(num_devices=NCORES)

    xT = nc.declare_dram_parameter("xT", [D, S], BF16, isOutput=False)
    wqT = nc.declare_dram_parameter("wqT", [D, R], BF16, isOutput=False)
    wkT = nc.declare_dram_parameter("wkT", [D, R], BF16, isOutput=False)
    wvT = nc.declare_dram_parameter("wvT", [D, 130], BF16, isOutput=False)
    bq = nc.declare_dram_parameter("bq", [R, 1], F32, isOutput=False)
    bk = nc.declare_dram_parameter("bk", [R, 1], F32, isOutput=False)
    bvb = nc.declare_dram_parameter("bvb", [128, 130], F32, isOutput=False)
    woT = nc.declare_dram_parameter("woT", [D, D], BF16, isOutput=False)
    bob = nc.declare_dram_parameter("bob", [128, D], F32, isOutput=False)
    maskt = nc.declare_dram_parameter("maskt", [128, 128], BF16, isOutput=False)
    ones = nc.declare_dram_parameter("ones", [1, HD], BF16, isOutput=False)
    out_ext = nc.declare_dram_parameter("out", [NT * 128, D], F32, isOutput=True)

    # AllToAll payload (one per q tile; ~15us each, pipelined 4 tiles deep
    # behind compute). Chunk j = this core's 128 head-dims for tile tokens
    # 64j..64(j+1), flattened [p, c] -> p*64+c; rec row i = the same from
    # global rank i (ranks 0-3 batch 0, 4-7 batch 1).
    GROUPS = [(t,) for t in range(NT)]
    TILE_FLAT = 128 * 64
    group_of = {t: gi for gi, g in enumerate(GROUPS) for t in g}
    parts = [nc.dram_tensor(f"part{gi}", [8, len(g) * TILE_FLAT], BF16)
             for gi, g in enumerate(GROUPS)]
    recs = [nc.dram_tensor(f"rec{gi}", [8, len(g) * TILE_FLAT], BF16)
            for gi, g in enumerate(GROUPS)]

    with tile.TileContext(nc) as tc:
        with (
            tc.tile_pool(name="const", bufs=1) as cpool,
            tc.tile_pool(name="xres", bufs=1) as xpool,
            tc.tile_pool(name="pt", bufs=12) as ppool,
            tc.tile_pool(name="small", bufs=3) as spool,
            tc.tile_pool(name="stage", bufs=3) as stpool,
            tc.tile_pool(name="ps_s", bufs=2, space="PSUM") as ps_s,
            tc.tile_pool(name="ps_o", bufs=1, space="PSUM") as ps_o,
            tc.tile_pool(name="ps_p", bufs=1, space="PSUM") as ps_p,
        ):
            # ---------- loads, in first-use order: x halves, then qkv
            # weights, then the epilogue constants
            xt = [xpool.tile([128, S], BF16, tag=f"xt{c}", name=f"xt{c}")
                  for c in range(4)]
            qT = xpool.tile([128, S], BF16, tag="qT")
            kT = xpool.tile([128, S], BF16, tag="kT")
            vaug = xpool.tile([128, 32 * 130], BF16, tag="vaug")

            def load_x(c, half):
                nc.sync.dma_start(
                    xt[c][:, (S // 2) * half:(S // 2) * (half + 1)],
                    xT[128 * c:128 * (c + 1),
                       (S // 2) * half:(S // 2) * (half + 1)],
                )

            for c in range(4):
                load_x(c, 0)
            wq_sb = cpool.tile([128, D], BF16)
            nc.sync.dma_start(wq_sb[:].rearrange("p (c m) -> p c m", c=4),
                              wqT[:, :].rearrange("(c p) m -> p c m", p=128))
            wk_sb = cpool.tile([128, D], BF16)
            nc.sync.dma_start(wk_sb[:].rearrange("p (c m) -> p c m", c=4),
                              wkT[:, :].rearrange("(c p) m -> p c m", p=128))
            wv_sb = cpool.tile([128, 4 * 130], BF16)
            nc.sync.dma_start(wv_sb[:].rearrange("p (c m) -> p c m", c=4),
                              wvT[:, :].rearrange("(c p) m -> p c m", p=128))
            bq_sb = cpool.tile([R, 1], F32)
            nc.sync.dma_start(bq_sb[:], bq[:, :])
            bk_sb = cpool.tile([R, 1], F32)
            nc.sync.dma_start(bk_sb[:], bk[:, :])
            bvb_sb = cpool.tile([128, 130], F32)
            nc.sync.dma_start(bvb_sb[:], bvb[:, :])
            mask_sb = cpool.tile([128, 128], BF16)
            nc.sync.dma_start(mask_sb[:], maskt[:, :])
            ones_sb = cpool.tile([1, HD], BF16)
            nc.sync.dma_start(ones_sb[:], ones[:, :])
            for c in range(4):
                load_x(c, 1)
            # O-proj weights needed only from the first epilogue onward
            wo_sb = cpool.tile([128, 4 * D], BF16)
            nc.sync.dma_start(wo_sb[:].rearrange("p (g m) -> p g m", g=4),
                              woT[:, :].rearrange("(g p) m -> p g m", p=128))
            bob_sb = cpool.tile([128, D], F32)
            nc.sync.dma_start(bob_sb[:], bob[:, :])

            def proj_units(nt):
                # 6 small work units (q, k, 4x v) for token block nt, fed
                # into the j-loop slack slot so exp never starves.
                def qk_unit(w_sb, b_sb, dst):
                    def u():
                        ps = ps_p.tile([128, QW], F32, tag="po", name="ps_qk")
                        for c in range(4):
                            nc.tensor.matmul(
                                ps[:],
                                w_sb[:, 128 * c:128 * (c + 1)],
                                xt[c][:, QW * nt:QW * (nt + 1)],
                                start=(c == 0), stop=(c == 3),
                            )
                        nc.vector.tensor_scalar_add(
                            dst[:, QW * nt:QW * (nt + 1)], ps[:], b_sb[:])
                    return u

                def v_unit(tb):
                    def u():
                        ps = ps_p.tile([128, QW], F32, tag="po", name="ps_v")
                        for c in range(4):
                            nc.tensor.matmul(
                                ps[:, 0:130],
                                xt[c][:, 128 * tb:128 * (tb + 1)],
                                wv_sb[:, 130 * c:130 * (c + 1)],
                                start=(c == 0), stop=(c == 3),
                            )
                        nc.vector.tensor_add(
                            vaug[:, 130 * tb:130 * (tb + 1)], ps[:, 0:130],
                            bvb_sb[:])
                    return u

                return ([qk_unit(wq_sb, bq_sb, qT), qk_unit(wk_sb, bk_sb, kT)]
                        + [v_unit(tb) for tb in range(4 * nt, 4 * (nt + 1))])

            def att_jloop(t, units):
                # `units`: deferred work (next tile's projections, old tiles'
                # O-proj) emitted between the s matmul and PV of a j
                # iteration — the tensor engine fills that slot while the
                # scalar engine runs exp, so exp never starves at tile
                # boundaries.
                nj = 4 * t + 4          # causal: k blocks 0 .. 4t+3
                o0 = ps_o.tile([128, QW], F32, tag="o0")
                o1 = ps_o.tile([128, QW], F32, tag="o1")
                n_units = len(units)
                emitted = 0
                for j in range(nj):
                    # causal: q columns < q0 are fully masked for this k block
                    q0 = max(0, 128 * (j - 4 * t))
                    s = ps_s.tile([128, 2 * QW], F32, tag="s")
                    for h in (0, 1):
                        nc.tensor.matmul(
                            s[:, QW * h + q0:QW * (h + 1)],
                            kT[64 * h:64 * (h + 1), 128 * j:128 * (j + 1)],
                            qT[64 * h:64 * (h + 1), QW * t + q0:QW * (t + 1)],
                            start=True, stop=True,
                        )
                    p = ppool.tile([128, 2 * QW], BF16, tag="p")
                    if q0 == 0:
                        nc.scalar.activation(p[:], s[:], AF.Exp, bias=0.0, scale=0.125)
                    else:
                        sv = s[:].rearrange("k (h q) -> k h q", h=2)[:, :, q0:QW]
                        pv = p[:].rearrange("k (h q) -> k h q", h=2)[:, :, q0:QW]
                        nc.scalar.activation(pv, sv, AF.Exp, bias=0.0, scale=0.125)
                    while emitted < n_units and emitted <= j * n_units // nj:
                        units[emitted]()
                        emitted += 1
                    if j >= 4 * t:  # diagonal 128-col boundary: 0/1 mask
                        for h in (0, 1):
                            nc.gpsimd.tensor_mul(
                                p[:, QW * h + q0:QW * h + q0 + 128],
                                p[:, QW * h + q0:QW * h + q0 + 128],
                                mask_sb[:, :],
                            )
                    for h, oo in ((0, o0), (1, o1)):
                        nc.tensor.matmul(
                            oo[0:65, q0:QW],
                            vaug[:, 130 * j + 65 * h:130 * j + 65 * (h + 1)],
                            p[:, QW * h + q0:QW * (h + 1)],
                            start=(j == 0), stop=(j == nj - 1),
                        )
                for u in units[emitted:]:   # leftovers (defensive)
                    u()
                return o0, o1

            def norm_send(t, o0, o1):
                # l rows (row 64 of o0/o1) -> SBUF (bf16), broadcast across
                # partitions via K=1 bf16 matmuls into a recycled s-tile,
                # reciprocal, then scale O^T and stage for the AllToAll.
                lrow = spool.tile([1, 2 * QW], BF16, tag="lrow")
                nc.vector.tensor_copy(lrow[0:1, 0:QW], o0[64:65, :])
                nc.vector.tensor_copy(lrow[0:1, QW:2 * QW], o1[64:65, :])
                lb = ps_s.tile([128, 2 * QW], F32, tag="s")
                for h in range(2):
                    nc.tensor.matmul(
                        lb[0:64, QW * h:QW * (h + 1)],
                        ones_sb[0:1, :],
                        lrow[0:1, QW * h:QW * (h + 1)],
                        start=True, stop=True,
                    )
                linv = spool.tile([64, 2 * QW], F32, tag="linv")
                nc.vector.reciprocal_approx_fast(linv[:], lb[0:64, :])
                ocn0 = spool.tile([64, QW], BF16, tag="ocn0")
                ocn1 = spool.tile([64, QW], BF16, tag="ocn1")
                nc.vector.tensor_mul(ocn0[:], o0[0:64, :], linv[:, 0:QW])
                nc.vector.tensor_mul(ocn1[:], o1[0:64, :], linv[:, QW:2 * QW])
                gi = group_of[t]
                base = GROUPS[gi].index(t) * TILE_FLAT
                for dims0, ocn in ((0, ocn0), (64 * 64, ocn1)):
                    nc.sync.dma_start(
                        parts[gi][:, base + dims0:base + dims0 + 64 * 64]
                        .rearrange("r (p c) -> p r c", p=64),
                        ocn[:].rearrange("p (r c) -> p r c", r=8),
                    )

            def launch_a2a(gi):
                nc.gpsimd.collective_compute(
                    "AllToAll",
                    mybir.AluOpType.bypass,
                    replica_groups=[[0, 1, 2, 3, 4, 5, 6, 7]],
                    ins=[parts[gi][:, :]],
                    outs=[recs[gi][:, :]],
                )

            def recv_oproj(t):
                # ysb cols 128g+[0:64] = batch-0 sender g, +[64:128] = batch-1
                # sender g: each 128-col block is one lhsT (same Wo rows).
                gi = group_of[t]
                base = GROUPS[gi].index(t) * TILE_FLAT
                ysb = stpool.tile([128, D], BF16, tag="ysb")
                for b in range(2):
                    nc.sync.dma_start(
                        ysb[:].rearrange("p (g b c) -> b p g c", g=4, b=2)[b],
                        recs[gi][4 * b:4 * (b + 1), base:base + TILE_FLAT]
                        .rearrange("g (p c) -> p g c", p=128),
                    )
                po = ps_p.tile([128, D], F32, tag="pr")
                for g in range(4):
                    nc.tensor.matmul(
                        po[:],
                        ysb[:, 128 * g:128 * (g + 1)],
                        wo_sb[:, D * g:D * (g + 1)],
                        start=(g == 0), stop=(g == 3),
                    )
                ost = stpool.tile([128, D], F32, tag="ost")
                nc.vector.tensor_add(ost[:], po[:], bob_sb[:])
                nc.sync.dma_start(out_ext[128 * t:128 * (t + 1), :], ost[:])

            for u in proj_units(0):
                u()
            for t in range(NT):
                units = []
                if t >= 6:
                    units.append(lambda tr=t - 6: recv_oproj(tr))
                if t + 1 < NT:
                    units.extend(proj_units(t + 1))
                o0, o1 = att_jloop(t, units)
                norm_send(t, o0, o1)
                launch_a2a(group_of[t])
            for tr in range(2, NT):
                recv_oproj(tr)

    nc.finalize()
    return nc


def _make_in_maps(x, Wqkv, bqkv, Wo, bo):
    # causal 0/1 multiplicative mask for the diagonal 128x128 sub-block:
    # keep (p, o) where o >= p (k = block_base + p, q = block_base + o)
    p_idx = np.arange(128)[:, None]
    o_idx = np.arange(128)[None, :]
    maskt = (o_idx >= p_idx).astype(np.float32).astype(BF16_NP)

    in_maps = []
    for core in range(NCORES):
        b = core // 4
        g = core % 4
        rows = slice(128 * g, 128 * (g + 1))
        wq = Wqkv[0:D][rows]            # [128, 512]
        wk = Wqkv[D:2 * D][rows]
        wv = Wqkv[2 * D:3 * D][rows]
        wvT = np.zeros((D, 130), dtype=np.float32)
        wvT[:, 0:64] = wv[0:64].T
        wvT[:, 65:129] = wv[64:128].T
        bvb = np.zeros((128, 130), dtype=np.float32)
        bvb[:, 0:64] = bqkv[2 * D:3 * D][rows][0:64][None, :]
        bvb[:, 64] = 1.0
        bvb[:, 65:129] = bqkv[2 * D:3 * D][rows][64:128][None, :]
        bvb[:, 129] = 1.0
        in_maps.append({
            "xT": np.ascontiguousarray(x[b].T).astype(BF16_NP),
            "wqT": np.ascontiguousarray(wq.T).astype(BF16_NP),
            "wkT": np.ascontiguousarray(wk.T).astype(BF16_NP),
            "wvT": wvT.astype(BF16_NP),
            "bq": np.ascontiguousarray(bqkv[0:D][rows][:, None]).astype(np.float32),
            "bk": np.ascontiguousarray(bqkv[D:2 * D][rows][:, None]).astype(np.float32),
            "bvb": bvb,
            "woT": np.ascontiguousarray(Wo.T).astype(BF16_NP),
            "bob": np.tile(bo.astype(np.float32)[None, :], (128, 1)),
            "maskt": maskt,
            "ones": np.ones((1, HD), dtype=BF16_NP),
        })
    return in_maps


def run(x, Wqkv, bqkv, Wo, bo, trace=False):
    if "nc" not in _CACHE:
        _CACHE["nc"] = _build_nc()
    nc = _CACHE["nc"]
    in_maps = _make_in_maps(x, Wqkv, bqkv, Wo, bo)
    res = run_bass_kernel_spmd(nc, in_maps, core_ids=list(range(NCORES)), trace=trace)
    out = np.empty((B, S, D), dtype=np.float32)
    for core in range(NCORES):
        o = res.results[core]["out"]
        # 8-way AllToAll: core j owns tile-t tokens 512t+64j..+64 for BOTH
        # batches (rows 128t..+64 = batch 0, rows 128t+64..+128 = batch 1)
        for t in range(NT):
            tok = QW * t + 64 * core
            out[0, tok:tok + 64, :] = o[128 * t:128 * t + 64]
            out[1, tok:tok + 64, :] = o[128 * t + 64:128 * (t + 1)]
    return out, res


def kernel(x, Wqkv, bqkv, Wo, bo):
    out, _ = run(np.asarray(x, dtype=np.float32), np.asarray(Wqkv, dtype=np.float32),
                 np.asarray(bqkv, dtype=np.float32), np.asarray(Wo, dtype=np.float32),
                 np.asarray(bo, dtype=np.float32))
    return out
